# revision 18
# baseline (speedup 1.0000x reference)
"""GNN message-passing kernel v2 for trn2 (8 cores).

h = relu(BN(s1 @ W_pre)); agg = segment_sum(h[src], dst);
out = relu((1-b)*support + b*support@W_op), support = 0.9*(h+agg) + 0.1*x_0.

v3 sharding: s1/x_0/dst sharded by node across 8 cores.
- P1: per-core z = s1_m @ W (bf16) chunk matmuls; column sums of z and z^2
  accumulated in PSUM via ones-lhsT matmuls.
- AR: AllReduce the [1,512] partial sums -> global BN stats; fold affine
  into W (scale) + bias row (K=1 matmul).
- P3: h_m = relu(s1_m @ Wsc + bias) -> AG input bounce (bf16, chunk-major).
- AG: AllGather h shards -> full gather table [1024, 49*256] in local DRAM.
- C: per dst-tile dma_gather of source rows + selection-matrix matmuls
  (segment sum) + epilogue (GCNII mix with folded W_op), as v1.
"""
import math
import numpy as np
import ml_dtypes

import concourse.bass as bass
import concourse.bacc as bacc
import concourse.mybir as mybir
from concourse.tile import TileContext

BF16 = mybir.dt.bfloat16
F32 = mybir.dt.float32
I16 = mybir.dt.int16
FP8 = mybir.dt.float8e4

ALPHA = 0.1
LAMBDA = 0.5
BN_EPS = 1e-5
BETA_C = float(np.log(LAMBDA / 1.0 + 1.0))   # 0.405465
W_OP_SCALE = BETA_C / (1.0 - BETA_C)
OUT_SCALE = 1.0 - BETA_C


class Prob:
    def __init__(self, N, E, C, HID, n_cores):
        self.N, self.E, self.C, self.HID, self.n_cores = N, E, C, HID, n_cores
        assert C == 256 and HID == 256
        self.shard = N // n_cores                      # nodes per core
        assert self.shard * n_cores == N
        self.tiles = math.ceil(self.shard / 128)       # chunks per shard (dst & src)
        self.shard_pad = self.tiles * 128
        # gather-table row space: row = (rank*128 + p)*tiles + c
        self.half_rows = (n_cores // 2) * 128 * self.tiles
        assert self.half_rows < 32768


def host_prep(prob, s1, x_0, edge_index):
    """Per-core input maps. All numpy."""
    p = prob
    N, E, M = p.N, p.E, p.n_cores
    s1 = np.asarray(s1, dtype=np.float32)
    x_0 = np.asarray(x_0, dtype=np.float32)
    src = np.asarray(edge_index[0], dtype=np.int64)
    dst = np.asarray(edge_index[1], dtype=np.int64)
    # self edges handled densely in-kernel (identity matmul on local h shard)

    core = dst // p.shard
    rel = dst - core * p.shard
    trel = rel >> 7
    prel = rel & 127
    # src -> gather-table row
    sr = src // p.shard
    sl = src - sr * p.shard
    sc = sl >> 7
    sp = sl & 127
    srow = (sr * 128 + sp) * p.tiles + sc
    half = (srow >= p.half_rows).astype(np.int64)
    srow = srow - half * p.half_rows

    gid = (core * p.tiles + trel) * 2 + half
    ngroups = M * p.tiles * 2
    order = np.argsort(gid, kind="stable")
    cnt = np.bincount(gid, minlength=ngroups).reshape(M, p.tiles, 2)
    K = np.maximum(np.ceil(cnt / 128.0).astype(np.int64).max(axis=0), 1)  # [tiles, 2]
    off = np.zeros((p.tiles, 2), np.int64)
    run = 0
    for t in range(p.tiles):
        for h in (0, 1):
            off[t, h] = run
            run += K[t, h]
    ktot = run
    srow_s = srow[order]
    drel_s = prel[order]
    gstart = np.zeros(ngroups + 1, np.int64)
    np.cumsum(np.bincount(gid[order], minlength=ngroups), out=gstart[1:])
    idx_flat = np.zeros((M, ktot * 128), np.int16)
    drel_flat = np.full((M, ktot * 128), 200.0, np.float32)
    for m in range(M):
        for t in range(p.tiles):
            for h in (0, 1):
                g = (m * p.tiles + t) * 2 + h
                a, b = gstart[g], gstart[g + 1]
                n = b - a
                base = off[t, h] * 128
                idx_flat[m, base:base + n] = srow_s[a:b].astype(np.int16)
                drel_flat[m, base:base + n] = drel_s[a:b].astype(np.float32)
    idx_lay = idx_flat.reshape(M, ktot * 8, 16).transpose(0, 2, 1)
    idx_lay = np.tile(idx_lay, (1, 8, 1))                           # [M, 128, ktot*8]
    drel_lay = np.ascontiguousarray(drel_flat.reshape(M, ktot, 128).transpose(0, 2, 1))

    # per-core s1 shard, transposed+padded: [C, shard_pad] bf16
    s1Ts = np.zeros((M, p.C, p.shard_pad), np.float32)
    for m in range(M):
        s1Ts[m, :, :p.shard] = s1[m * p.shard:(m + 1) * p.shard].T
    s1Ts = s1Ts.astype(ml_dtypes.bfloat16)

    x0s = np.zeros((M, p.shard_pad, p.HID), np.float32)
    for m in range(M):
        x0s[m, :p.shard] = ALPHA * x_0[m * p.shard:(m + 1) * p.shard]
    x0s = x0s.reshape(M, p.tiles, 128, p.HID).transpose(0, 2, 1, 3).reshape(M, 128, -1)
    x0s = np.ascontiguousarray(x0s)

    iota = np.broadcast_to(np.arange(128, dtype=np.float32), (128, 128)).astype(ml_dtypes.bfloat16).copy()
    ident = np.eye(128, dtype=np.float32)
    ones1 = np.ones((1, 128), np.float32)
    onesc = np.ones((128, 1), np.float32)

    meta = dict(K=K, off=off, ktot=ktot)
    shared = dict(iota=iota, ident=ident, ones1=ones1, onesc=onesc)
    in_maps = []
    for m in range(M):
        d = dict(shared)
        d["s1T"] = np.ascontiguousarray(s1Ts[m])
        d["idxall"] = idx_lay[m]
        d["drel"] = drel_lay[m]
        d["x0s"] = x0s[m]
        in_maps.append(d)
    return in_maps, meta


def build_kernel(prob, meta, W_pre, gamma, beta_bn, W_op, nloop=1, nq=4,
                 phases="13C", c_parts="gse", coll=True):
    p = prob
    K, off, ktot = meta["K"], meta["off"], meta["ktot"]
    C, HID = p.C, p.HID
    M = p.n_cores
    cpr = p.tiles                  # chunks per shard
    nc = bacc.Bacc("TRN2", target_bir_lowering=False, debug=False,
                   num_devices=M, num_swdge_queues=nq)
    t_s1T = nc.dram_tensor("s1T", [C, p.shard_pad], BF16, kind="ExternalInput")
    t_wpre = nc.dram_tensor("wpre", [C, HID], F32, kind="ExternalInput")
    t_gamma = nc.dram_tensor("gamma", [1, HID], F32, kind="ExternalInput")
    t_beta = nc.dram_tensor("beta", [1, HID], F32, kind="ExternalInput")
    t_wop = nc.dram_tensor("wop", [HID, HID], F32, kind="ExternalInput")
    t_x0 = nc.dram_tensor("x0s", [128, cpr * HID], F32, kind="ExternalInput")
    t_idx = nc.dram_tensor("idxall", [128, ktot * 8], I16, kind="ExternalInput")
    t_drel = nc.dram_tensor("drel", [128, ktot], F32, kind="ExternalInput")
    t_iota = nc.dram_tensor("iota", [128, 128], BF16, kind="ExternalInput")
    t_ident = nc.dram_tensor("ident", [128, 128], F32, kind="ExternalInput")
    t_ones1 = nc.dram_tensor("ones1", [1, 128], F32, kind="ExternalInput")
    t_onesc = nc.dram_tensor("onesc", [128, 1], F32, kind="ExternalInput")
    t_out = nc.dram_tensor("out", [128, cpr * HID], F32, kind="ExternalOutput")
    # collective buffers
    t_arin = nc.dram_tensor("arin", [1, 2 * HID], F32)
    t_arout = nc.dram_tensor("arout", [1, 2 * HID], F32)
    t_hb = nc.dram_tensor("hb", [128, cpr * HID], BF16)
    t_ag = nc.dram_tensor("agh", [M * 128, cpr * HID], BF16)
    groups = [list(range(M))]

    def loop(tc, body, active):
        if active and nloop > 1:
            with tc.For_i(0, nloop, 1):
                body()
        else:
            body()

    tc1 = TileContext(nc)
    with tc1 as tc:
        with (tc.tile_pool(name="const", bufs=1) as cpool,
              tc.tile_pool(name="work", bufs=5) as wpool,
              tc.tile_pool(name="hout", bufs=4) as hpool,
              tc.tile_pool(name="psZ", bufs=4, space="PSUM") as psZ,
              tc.tile_pool(name="psB", bufs=1, space="PSUM") as psB,
              tc.tile_pool(name="psS", bufs=1, space="PSUM") as psS,
              tc.tile_pool(name="small", bufs=1) as smpool):
            # ---- constants ----
            s1sb = []
            for r in range(2):
                t = cpool.tile([128, p.shard_pad], BF16, tag=f"s1{r}")
                nc.sync.dma_start(out=t[:], in_=t_s1T[r * 128:(r + 1) * 128, :])
                s1sb.append(t)
            w_f32, w_bf = [], []
            for r in range(2):
                w = cpool.tile([128, HID], F32, tag=f"wf{r}")
                nc.sync.dma_start(out=w[:], in_=t_wpre[r * 128:(r + 1) * 128, :])
                w_f32.append(w)
                wb = cpool.tile([128, HID], BF16, tag=f"wb{r}")
                nc.vector.tensor_copy(out=wb[:], in_=w[:])
                w_bf.append(wb)
            gamma_sb = cpool.tile([1, HID], F32, tag="gm")
            nc.sync.dma_start(out=gamma_sb[:], in_=t_gamma[:])
            beta_sb = cpool.tile([1, HID], F32, tag="bt")
            nc.sync.dma_start(out=beta_sb[:], in_=t_beta[:])
            ones1_sb = cpool.tile([1, 128], F32, tag="on")
            nc.sync.dma_start(out=ones1_sb[:], in_=t_ones1[:])
            onesc_sb = cpool.tile([128, 1], F32, tag="onc")
            nc.sync.dma_start(out=onesc_sb[:], in_=t_onesc[:])
            onesc_bf = cpool.tile([128, 1], BF16, tag="oncb")
            nc.vector.tensor_copy(out=onesc_bf[:], in_=onesc_sb[:])
            ones1_bf = cpool.tile([1, 128], BF16, tag="on16")
            nc.vector.tensor_copy(out=ones1_bf[:], in_=ones1_sb[:])

            # ---- P1: z -> SBUF store; then squares + column-sum matmuls ----
            sums = psS.tile([1, 2 * HID], F32, tag="sums", name="sums")
            zst = cpool.tile([128, cpr * HID], BF16, tag="zst")
            zqt = cpool.tile([128, cpr * HID], BF16, tag="zqt")
            def phase1():
                for j in range(cpr):
                    zc = psZ.tile([128, HID], F32, tag="zc")
                    co = j * 128
                    nc.tensor.matmul(zc[:], lhsT=s1sb[0][:, co:co + 128],
                                     rhs=w_bf[0][:], start=True, stop=False)
                    nc.tensor.matmul(zc[:], lhsT=s1sb[1][:, co:co + 128],
                                     rhs=w_bf[1][:], start=False, stop=True)
                    nc.vector.tensor_copy(out=zst[:, j * HID:(j + 1) * HID],
                                          in_=zc[:])
                for j in range(cpr):
                    nc.scalar.activation(out=zqt[:, j * HID:(j + 1) * HID],
                                         in_=zst[:, j * HID:(j + 1) * HID],
                                         func=mybir.ActivationFunctionType.Square,
                                         bias=0.0, scale=1.0)
                for j in range(cpr):
                    nc.tensor.matmul(sums[:, :HID], lhsT=onesc_bf[:],
                                     rhs=zst[:, j * HID:(j + 1) * HID],
                                     start=(j == 0), stop=(j == cpr - 1))
                    nc.tensor.matmul(sums[:, HID:], lhsT=onesc_bf[:],
                                     rhs=zqt[:, j * HID:(j + 1) * HID],
                                     start=(j == 0), stop=(j == cpr - 1))
            loop(tc, phase1, "1" in phases)

            # ---- AR: allreduce stats ----
            sums_sb = smpool.tile([1, 2 * HID], F32, tag="ssb")
            nc.vector.tensor_copy(out=sums_sb[:], in_=sums[:])
            nc.sync.dma_start(out=t_arin[:], in_=sums_sb[:])
            def phase_ar():
                nc.gpsimd.collective_compute(
                    "AllReduce", mybir.AluOpType.add, replica_groups=groups,
                    ins=[t_arin[:]], outs=[t_arout[:]])
            if coll:
                loop(tc, phase_ar, "r" in phases)
            stats_sb = smpool.tile([1, 2 * HID], F32, tag="stats")
            nc.sync.dma_start(out=stats_sb[:],
                              in_=(t_arout[:] if coll else t_arin[:]))

            # ---- stats finalize ----
            invn = 1.0 / p.N
            mu = smpool.tile([1, HID], F32, tag="mu")
            nc.vector.tensor_scalar(out=mu[:], in0=stats_sb[:, :HID], scalar1=invn,
                                    scalar2=None, op0=mybir.AluOpType.mult)
            var = smpool.tile([1, HID], F32, tag="var")
            nc.vector.tensor_scalar(out=var[:], in0=stats_sb[:, HID:], scalar1=invn,
                                    scalar2=None, op0=mybir.AluOpType.mult)
            musq = smpool.tile([1, HID], F32, tag="musq")
            nc.vector.tensor_tensor(out=musq[:], in0=mu[:], in1=mu[:],
                                    op=mybir.AluOpType.mult)
            nc.vector.tensor_tensor(out=var[:], in0=var[:], in1=musq[:],
                                    op=mybir.AluOpType.subtract)
            nc.vector.tensor_scalar(out=var[:], in0=var[:], scalar1=BN_EPS,
                                    scalar2=None, op0=mybir.AluOpType.add)
            sq = smpool.tile([1, HID], F32, tag="sq")
            nc.scalar.activation(out=sq[:], in_=var[:],
                                 func=mybir.ActivationFunctionType.Sqrt,
                                 bias=0.0, scale=1.0)
            rs = smpool.tile([1, HID], F32, tag="rs")
            nc.vector.reciprocal(out=rs[:], in_=sq[:])
            a_vec = smpool.tile([1, HID], F32, tag="av")
            nc.vector.tensor_tensor(out=a_vec[:], in0=rs[:], in1=gamma_sb[:],
                                    op=mybir.AluOpType.mult)
            b_vec = smpool.tile([1, HID], F32, tag="bv")
            nc.vector.tensor_tensor(out=b_vec[:], in0=mu[:], in1=a_vec[:],
                                    op=mybir.AluOpType.mult)
            nc.vector.tensor_tensor(out=b_vec[:], in0=beta_sb[:], in1=b_vec[:],
                                    op=mybir.AluOpType.subtract)
            ab_cat = smpool.tile([1, 2 * HID], F32, tag="abc")
            nc.vector.tensor_copy(out=ab_cat[:, :HID], in_=a_vec[:])
            nc.vector.tensor_copy(out=ab_cat[:, HID:], in_=b_vec[:])
            ps_ab = psB.tile([128, 2 * HID], F32, tag="zab", name="ps_ab")
            nc.tensor.matmul(ps_ab[:], lhsT=ones1_sb[:], rhs=ab_cat[:],
                             start=True, stop=True)
            a_bc = cpool.tile([128, HID], F32, tag="abc2")
            nc.vector.tensor_copy(out=a_bc[:], in_=ps_ab[:, :HID])
            b_bc = cpool.tile([128, HID], F32, tag="bbc2")
            nc.vector.tensor_copy(out=b_bc[:], in_=ps_ab[:, HID:])

            # ---- P3: h = relu(a*z + b) from stored z (DVE/Act only) ----
            HSPAN = 7
            def phase3():
                for j in range(cpr):
                    zj = zst[:, j * HID:(j + 1) * HID]
                    t1 = wpool.tile([128, HID], F32, tag="t1")
                    nc.vector.tensor_tensor(out=t1[:], in0=zj, in1=a_bc[:],
                                            op=mybir.AluOpType.mult)
                    hs = j // HSPAN
                    ho = j % HSPAN
                    he = min(cpr, (hs + 1) * HSPAN) - hs * HSPAN
                    if ho == 0:
                        hsp = hpool.tile([128, HSPAN * HID], BF16, tag="hsp",
                                         name=f"hsp_{hs % 4}")
                        phase3.hsp = hsp
                    hsp = phase3.hsp
                    hj = hsp[:, ho * HID:(ho + 1) * HID]
                    t2 = wpool.tile([128, HID], F32, tag="t2")
                    if j % 2 == 0:
                        nc.vector.tensor_tensor(out=t2[:], in0=t1[:], in1=b_bc[:],
                                                op=mybir.AluOpType.add)
                        nc.vector.tensor_scalar(out=hj, in0=t2[:], scalar1=0.0,
                                                scalar2=None,
                                                op0=mybir.AluOpType.max)
                    else:
                        nc.any.tensor_tensor(out=t2[:], in0=t1[:], in1=b_bc[:],
                                             op=mybir.AluOpType.add)
                        nc.scalar.activation(out=hj, in_=t2[:],
                                             func=mybir.ActivationFunctionType.Relu,
                                             bias=0.0, scale=1.0)
                    if ho == he - 1:
                        nc.sync.dma_start(
                            out=t_hb[:, hs * HSPAN * HID:(hs * HSPAN + he) * HID],
                            in_=hsp[:, :he * HID])
            loop(tc, phase3, "3" in phases)

            # ---- AG: allgather h shards ----
            def phase_ag():
                nc.gpsimd.collective_compute(
                    "AllGather", mybir.AluOpType.bypass, replica_groups=groups,
                    ins=[t_hb[:]], outs=[t_ag[:]])
            if coll:
                loop(tc, phase_ag, "G" in phases)

        # ---------------- phase C: aggregate + output ----------------
        with (tc.tile_pool(name="c2", bufs=1) as cpool,
              tc.tile_pool(name="gat", bufs=10) as gpool,
              tc.tile_pool(name="sel", bufs=18) as selp,
              tc.tile_pool(name="epi", bufs=3) as epool,
              tc.tile_pool(name="psG", bufs=4, space="PSUM") as psG,
              tc.tile_pool(name="psT", bufs=2, space="PSUM") as psT,
              tc.tile_pool(name="psO", bufs=2, space="PSUM") as psO):
            idx_sb = cpool.tile([128, ktot * 8], I16, tag="idx")
            nc.sync.dma_start(out=idx_sb[:], in_=t_idx[:])
            drel_sb = cpool.tile([128, ktot], F32, tag="dr")
            nc.sync.dma_start(out=drel_sb[:], in_=t_drel[:])
            iota_sb = cpool.tile([128, 128], BF16, tag="io")
            nc.sync.dma_start(out=iota_sb[:], in_=t_iota[:])
            ident_sb = cpool.tile([128, 128], F32, tag="idn")
            nc.sync.dma_start(out=ident_sb[:], in_=t_ident[:])
            ident_bf = cpool.tile([128, 128], BF16, tag="idnb")
            nc.vector.tensor_copy(out=ident_bf[:], in_=ident_sb[:])
            hsh = cpool.tile([128, cpr * HID], BF16, tag="hsh")
            nc.sync.dma_start(out=hsh[:], in_=t_hb[:])
            wop2 = []
            for r in range(2):
                w = cpool.tile([128, HID], F32, tag=f"wo2{r}")
                nc.sync.dma_start(out=w[:], in_=t_wop[r * 128:(r + 1) * 128, :])
                wb = cpool.tile([128, HID], BF16, tag=f"wo2b{r}")
                nc.vector.tensor_scalar(out=wb[:], in0=w[:], scalar1=W_OP_SCALE,
                                        scalar2=None, op0=mybir.AluOpType.mult)
                wop2.append(wb)

            half_parts = (M // 2) * 128
            tbl_lo = t_ag[0:half_parts, :]
            tbl_hi = t_ag[half_parts:2 * half_parts, :]

            qn = [0]
            ESPAN = 7
            nspan = math.ceil(p.tiles / ESPAN)
            sup_sp = [epool.tile([128, ESPAN * HID], F32, tag="sup",
                                 name=f"sup{es}") for es in range(nspan)]
            def phaseC():
                # loop 1: gather + segment-sum; agg -> sup spans (Act, x0.9)
                for t in range(p.tiles):
                    gt = {}
                    for hh in (0, 1) if 'g' in c_parts else ():
                        kk = int(K[t, hh])
                        g = gpool.tile([128, kk * HID], BF16, tag=f"g{hh}")
                        tbl = tbl_lo if hh == 0 else tbl_hi
                        o8 = int(off[t, hh]) * 8
                        nc.gpsimd.dma_gather(
                            out_ap=g[:].rearrange("p (c d) -> p c d", d=HID),
                            in_ap=tbl.rearrange("q (c d) -> (q c) d", d=HID),
                            idxs_ap=idx_sb[:, o8:o8 + kk * 8],
                            num_idxs=kk * 128, num_idxs_reg=kk * 128,
                            elem_size=HID, single_packet=False,
                            queue_num=qn[0] % 4)
                        qn[0] += 1
                        gt[hh] = g
                    agg = psG.tile([128, HID], F32, tag="agg")
                    nmm = int(K[t, 0] + K[t, 1]) + 1
                    ci = 0
                    if 's' not in c_parts or 'g' not in c_parts:
                        nc.vector.memset(agg[:], 0.0)
                    else:
                        # self term: agg += h_shard chunk t (exact bf16)
                        nc.tensor.matmul(agg[:], lhsT=ident_bf[:],
                                         rhs=hsh[:, t * HID:(t + 1) * HID],
                                         start=True, stop=False)
                        ci = 1
                    for hh in ((0, 1) if ('s' in c_parts and 'g' in c_parts) else ()):
                        kk = int(K[t, hh])
                        for c in range(kk):
                            col = int(off[t, hh]) + c
                            S = selp.tile([128, 128], BF16)
                            nc.vector.tensor_scalar(
                                out=S[:], in0=iota_sb[:],
                                scalar1=drel_sb[:, col:col + 1], scalar2=None,
                                op0=mybir.AluOpType.is_equal)
                            nc.tensor.matmul(agg[:], lhsT=S[:],
                                             rhs=gt[hh][:, c * HID:(c + 1) * HID],
                                             start=(ci == 0), stop=(ci == nmm - 1))
                            ci += 1
                    if 'e' not in c_parts:
                        continue
                    es, eo = t // ESPAN, t % ESPAN
                    nc.scalar.activation(out=sup_sp[es][:, eo * HID:(eo + 1) * HID],
                                         in_=agg[:],
                                         func=mybir.ActivationFunctionType.Copy,
                                         bias=0.0, scale=(1.0 - ALPHA))
                    e1 = min(p.tiles, (es + 1) * ESPAN)
                    if t == e1 - 1:
                        epilogue_span(es)
                if 'e' not in c_parts:
                    return
            def epilogue_span(es):
                e0 = es * ESPAN
                e1 = min(p.tiles, e0 + ESPAN)
                ne = e1 - e0
                x0sp = epool.tile([128, ESPAN * HID], F32, tag="x0sp",
                                  name=f"x0sp{es % 2}")
                nc.sync.dma_start(out=x0sp[:, :ne * HID],
                                  in_=t_x0[:, e0 * HID:e1 * HID])
                outsp = epool.tile([128, ESPAN * HID], F32, tag="outsp",
                                   name=f"outsp{es % 2}")
                sup = sup_sp[es]
                nc.vector.tensor_tensor(out=sup[:, :ne * HID],
                                        in0=sup[:, :ne * HID],
                                        in1=x0sp[:, :ne * HID],
                                        op=mybir.AluOpType.add)
                for t in range(e0, e1):
                    eo = t - e0
                    st = sup[:, eo * HID:(eo + 1) * HID]
                    trp = psT.tile([128, HID], F32, tag="tr")
                    for r in range(2):
                        nc.tensor.transpose(
                            out=trp[:, r * 128:(r + 1) * 128],
                            in_=st[:, r * 128:(r + 1) * 128],
                            identity=ident_sb[:])
                    supT = epool.tile([128, HID], BF16, tag="supT")
                    nc.any.tensor_copy(out=supT[:], in_=trp[:])
                    ops = psO.tile([128, HID], F32, tag="o")
                    nc.tensor.matmul(ops[:], lhsT=supT[:, :128], rhs=wop2[0][:],
                                     start=True, stop=False)
                    nc.tensor.matmul(ops[:], lhsT=supT[:, 128:], rhs=wop2[1][:],
                                     start=False, stop=True)
                    ut = epool.tile([128, HID], F32, tag="u")
                    nc.vector.tensor_tensor(out=ut[:], in0=st, in1=ops[:],
                                            op=mybir.AluOpType.add)
                    nc.scalar.activation(out=outsp[:, eo * HID:(eo + 1) * HID],
                                         in_=ut[:],
                                         func=mybir.ActivationFunctionType.Relu,
                                         bias=0.0, scale=OUT_SCALE)
                nc.sync.dma_start(out=t_out[:, e0 * HID:e1 * HID],
                                  in_=outsp[:, :ne * HID])
            loop(tc, phaseC, "C" in phases)

    nc.compile()
    return nc


def make_weight_inputs(prob, W_pre, gamma, beta_bn, W_op):
    return dict(
        wpre=np.asarray(W_pre, np.float32),
        gamma=np.asarray(gamma, np.float32).reshape(1, -1),
        beta=np.asarray(beta_bn, np.float32).reshape(1, -1),
        wop=np.asarray(W_op, np.float32),
    )


def unpack_out(prob, arr):
    return arr.reshape(128, prob.tiles, prob.HID).transpose(1, 0, 2).reshape(
        prob.shard_pad, prob.HID)


# ======================================================================
# Self-contained execution via PJRT (axon)
# ======================================================================
import jax
from jax.sharding import Mesh, PartitionSpec, NamedSharding
from jax.experimental.shard_map import shard_map
from concourse.bass2jax import _bass_exec_p, install_neuronx_cc_hook, partition_id_tensor


def _build_exec(nc, n_cores):
    install_neuronx_cc_hook()
    partition_name = nc.partition_id_tensor.name if nc.partition_id_tensor else None
    in_names, out_names, out_avals, zero_outs = [], [], [], []
    for alloc in nc.m.functions[0].allocations:
        if not isinstance(alloc, mybir.MemoryLocationSet):
            continue
        name = alloc.memorylocations[0].name
        if alloc.kind == "ExternalInput":
            if name != partition_name:
                in_names.append(name)
        elif alloc.kind == "ExternalOutput":
            shape = tuple(alloc.tensor_shape)
            dtype = mybir.dt.np(alloc.dtype)
            out_names.append(name)
            out_avals.append(jax.core.ShapedArray(shape, dtype))
            zero_outs.append(np.zeros(shape, dtype))
    n_params = len(in_names)
    n_outs = len(out_avals)
    all_in_names = list(in_names) + list(out_names)
    if partition_name is not None:
        all_in_names.append(partition_name)

    def _body(*args):
        operands = list(args)
        if partition_name is not None:
            operands.append(partition_id_tensor())
        outs = _bass_exec_p.bind(
            *operands, out_avals=tuple(out_avals), in_names=tuple(all_in_names),
            out_names=tuple(out_names), lowering_input_output_aliases=(),
            sim_require_finite=True, sim_require_nnan=True, nc=nc)
        return tuple(outs)

    devices = jax.devices()[:n_cores]
    mesh = Mesh(np.asarray(devices), ("core",))
    in_specs = (PartitionSpec("core"),) * (n_params + n_outs)
    out_specs = (PartitionSpec("core"),) * n_outs
    donate = tuple(range(n_params, n_params + n_outs))
    fn = jax.jit(shard_map(_body, mesh=mesh, in_specs=in_specs,
                           out_specs=out_specs, check_rep=False),
                 donate_argnums=donate, keep_unused=True)
    return dict(fn=fn, in_names=in_names, out_names=out_names,
                out_avals=out_avals, zero_outs=zero_outs, mesh=mesh,
                n_cores=n_cores)


def _place_inputs(ex, in_maps):
    sh = NamedSharding(ex["mesh"], PartitionSpec("core"))
    n_cores = ex["n_cores"]
    return [jax.device_put(
        np.concatenate([np.asarray(in_maps[c][name]) for c in range(n_cores)], axis=0), sh)
        for name in ex["in_names"]]


def _run(ex, dev_in):
    sh = NamedSharding(ex["mesh"], PartitionSpec("core"))
    n_cores = ex["n_cores"]
    zs = [jax.device_put(np.zeros((n_cores * z.shape[0], *z.shape[1:]), z.dtype), sh)
          for z in ex["zero_outs"]]
    outs = jax.block_until_ready(ex["fn"](*dev_in, *zs))
    return [
        {name: np.asarray(outs[i]).reshape(n_cores, *ex["out_avals"][i].shape)[c]
         for i, name in enumerate(ex["out_names"])}
        for c in range(n_cores)
    ]


_CACHE = {}


def _get_compiled(prob, meta, W_pre, gamma, beta_bn, W_op, key):
    if key not in _CACHE:
        nc = build_kernel(prob, meta, W_pre, gamma, beta_bn, W_op, nloop=1)
        _CACHE[key] = _build_exec(nc, prob.n_cores)
    return _CACHE[key]


def kernel(s0=None, s1=None, x_0=None, W_pre=None, gamma=None, beta_bn=None,
           W_op=None, edge_index=None, drop_prob=None, training=None, **_ignored):
    s1 = np.asarray(s1, np.float32)
    x_0 = np.asarray(x_0, np.float32)
    W_pre = np.asarray(W_pre, np.float32)
    gamma = np.asarray(gamma, np.float32)
    beta_bn = np.asarray(beta_bn, np.float32)
    W_op = np.asarray(W_op, np.float32)
    edge_index = np.asarray(edge_index)
    N, C = s1.shape
    HID = W_pre.shape[1]
    E = edge_index.shape[1]
    prob = Prob(N, E, C, HID, n_cores=8)
    in_maps, meta = host_prep(prob, s1, x_0, edge_index)
    key = (N, E, C, HID, int(np.int64(edge_index[:, ::97]).sum()), meta["ktot"])
    ex = _get_compiled(prob, meta, W_pre, gamma, beta_bn, W_op, key)
    wins = make_weight_inputs(prob, W_pre, gamma, beta_bn, W_op)
    full_maps = [{**m, **wins} for m in in_maps]
    dev_in = _place_inputs(ex, full_maps)
    res = _run(ex, dev_in)
    out = np.concatenate(
        [unpack_out(prob, res[m]["out"])[:prob.shard] for m in range(prob.n_cores)],
        axis=0)
    return np.ascontiguousarray(out[:N]).astype(np.float32)


# revision 19
# speedup vs baseline: 1.0486x; 1.0486x over previous
"""GNN message-passing kernel v2 for trn2 (8 cores).

h = relu(BN(s1 @ W_pre)); agg = segment_sum(h[src], dst);
out = relu((1-b)*support + b*support@W_op), support = 0.9*(h+agg) + 0.1*x_0.

v3 sharding: s1/x_0/dst sharded by node across 8 cores.
- P1: per-core z = s1_m @ W (bf16) chunk matmuls; column sums of z and z^2
  accumulated in PSUM via ones-lhsT matmuls.
- AR: AllReduce the [1,512] partial sums -> global BN stats; fold affine
  into W (scale) + bias row (K=1 matmul).
- P3: h_m = relu(s1_m @ Wsc + bias) -> AG input bounce (bf16, chunk-major).
- AG: AllGather h shards -> full gather table [1024, 49*256] in local DRAM.
- C: per dst-tile dma_gather of source rows + selection-matrix matmuls
  (segment sum) + epilogue (GCNII mix with folded W_op), as v1.
"""
import math
import numpy as np
import ml_dtypes

import concourse.bass as bass
import concourse.bacc as bacc
import concourse.mybir as mybir
from concourse.tile import TileContext

BF16 = mybir.dt.bfloat16
F32 = mybir.dt.float32
I16 = mybir.dt.int16
FP8 = mybir.dt.float8e4

ALPHA = 0.1
LAMBDA = 0.5
BN_EPS = 1e-5
BETA_C = float(np.log(LAMBDA / 1.0 + 1.0))   # 0.405465
W_OP_SCALE = BETA_C / (1.0 - BETA_C)
OUT_SCALE = 1.0 - BETA_C


class Prob:
    def __init__(self, N, E, C, HID, n_cores):
        self.N, self.E, self.C, self.HID, self.n_cores = N, E, C, HID, n_cores
        assert C == 256 and HID == 256
        self.shard = N // n_cores                      # nodes per core
        assert self.shard * n_cores == N
        self.tiles = math.ceil(self.shard / 128)       # chunks per shard (dst & src)
        self.shard_pad = self.tiles * 128
        # gather-table row space: row = (rank*128 + p)*tiles + c
        self.half_rows = (n_cores // 2) * 128 * self.tiles
        assert self.half_rows < 32768


def host_prep(prob, s1, x_0, edge_index):
    """Per-core input maps. All numpy."""
    p = prob
    N, E, M = p.N, p.E, p.n_cores
    s1 = np.asarray(s1, dtype=np.float32)
    x_0 = np.asarray(x_0, dtype=np.float32)
    src = np.asarray(edge_index[0], dtype=np.int64)
    dst = np.asarray(edge_index[1], dtype=np.int64)
    # self edges handled densely in-kernel (identity matmul on local h shard)

    core = dst // p.shard
    rel = dst - core * p.shard
    trel = rel >> 7
    prel = rel & 127
    # src -> gather-table row
    sr = src // p.shard
    sl = src - sr * p.shard
    sc = sl >> 7
    sp = sl & 127
    srow = (sr * 128 + sp) * p.tiles + sc
    half = (srow >= p.half_rows).astype(np.int64)
    srow = srow - half * p.half_rows

    gid = (core * p.tiles + trel) * 2 + half
    ngroups = M * p.tiles * 2
    order = np.argsort(gid, kind="stable")
    cnt = np.bincount(gid, minlength=ngroups).reshape(M, p.tiles, 2)
    K = np.maximum(np.ceil(cnt / 128.0).astype(np.int64).max(axis=0), 1)  # [tiles, 2]
    off = np.zeros((p.tiles, 2), np.int64)
    run = 0
    for t in range(p.tiles):
        for h in (0, 1):
            off[t, h] = run
            run += K[t, h]
    ktot = run
    srow_s = srow[order]
    drel_s = prel[order]
    gstart = np.zeros(ngroups + 1, np.int64)
    np.cumsum(np.bincount(gid[order], minlength=ngroups), out=gstart[1:])
    idx_flat = np.zeros((M, ktot * 128), np.int16)
    drel_flat = np.full((M, ktot * 128), 200.0, np.float32)
    for m in range(M):
        for t in range(p.tiles):
            for h in (0, 1):
                g = (m * p.tiles + t) * 2 + h
                a, b = gstart[g], gstart[g + 1]
                n = b - a
                base = off[t, h] * 128
                idx_flat[m, base:base + n] = srow_s[a:b].astype(np.int16)
                drel_flat[m, base:base + n] = drel_s[a:b].astype(np.float32)
    idx_lay = idx_flat.reshape(M, ktot * 8, 16).transpose(0, 2, 1)
    idx_lay = np.tile(idx_lay, (1, 8, 1))                           # [M, 128, ktot*8]
    drel_lay = np.ascontiguousarray(drel_flat.reshape(M, ktot, 128).transpose(0, 2, 1))

    # per-core s1 shard, transposed+padded: [C, shard_pad] bf16
    s1Ts = np.zeros((M, p.C, p.shard_pad), np.float32)
    for m in range(M):
        s1Ts[m, :, :p.shard] = s1[m * p.shard:(m + 1) * p.shard].T
    s1Ts = s1Ts.astype(ml_dtypes.bfloat16)

    x0s = np.zeros((M, p.shard_pad, p.HID), np.float32)
    for m in range(M):
        x0s[m, :p.shard] = ALPHA * x_0[m * p.shard:(m + 1) * p.shard]
    x0s = x0s.reshape(M, p.tiles, 128, p.HID).transpose(0, 2, 1, 3).reshape(M, 128, -1)
    x0s = np.ascontiguousarray(x0s)

    iota = np.broadcast_to(np.arange(128, dtype=np.float32), (128, 128)).astype(ml_dtypes.bfloat16).copy()
    ident = np.eye(128, dtype=np.float32)
    ones1 = np.ones((1, 128), np.float32)
    onesc = np.ones((128, 1), np.float32)

    meta = dict(K=K, off=off, ktot=ktot)
    shared = dict(iota=iota, ident=ident, ones1=ones1, onesc=onesc)
    in_maps = []
    for m in range(M):
        d = dict(shared)
        d["s1T"] = np.ascontiguousarray(s1Ts[m])
        d["idxall"] = idx_lay[m]
        d["drel"] = drel_lay[m]
        d["x0s"] = x0s[m]
        in_maps.append(d)
    return in_maps, meta


def build_kernel(prob, meta, W_pre, gamma, beta_bn, W_op, nloop=1, nq=4,
                 phases="13C", c_parts="gse", coll=True):
    p = prob
    K, off, ktot = meta["K"], meta["off"], meta["ktot"]
    C, HID = p.C, p.HID
    M = p.n_cores
    cpr = p.tiles                  # chunks per shard
    nc = bacc.Bacc("TRN2", target_bir_lowering=False, debug=False,
                   num_devices=M, num_swdge_queues=nq)
    t_s1T = nc.dram_tensor("s1T", [C, p.shard_pad], BF16, kind="ExternalInput")
    t_wpre = nc.dram_tensor("wpre", [C, HID], F32, kind="ExternalInput")
    t_gamma = nc.dram_tensor("gamma", [1, HID], F32, kind="ExternalInput")
    t_beta = nc.dram_tensor("beta", [1, HID], F32, kind="ExternalInput")
    t_wop = nc.dram_tensor("wop", [HID, HID], F32, kind="ExternalInput")
    t_x0 = nc.dram_tensor("x0s", [128, cpr * HID], F32, kind="ExternalInput")
    t_idx = nc.dram_tensor("idxall", [128, ktot * 8], I16, kind="ExternalInput")
    t_drel = nc.dram_tensor("drel", [128, ktot], F32, kind="ExternalInput")
    t_iota = nc.dram_tensor("iota", [128, 128], BF16, kind="ExternalInput")
    t_ident = nc.dram_tensor("ident", [128, 128], F32, kind="ExternalInput")
    t_ones1 = nc.dram_tensor("ones1", [1, 128], F32, kind="ExternalInput")
    t_onesc = nc.dram_tensor("onesc", [128, 1], F32, kind="ExternalInput")
    t_out = nc.dram_tensor("out", [128, cpr * HID], F32, kind="ExternalOutput")
    # collective buffers
    t_arin = nc.dram_tensor("arin", [1, 2 * HID], F32)
    t_arout = nc.dram_tensor("arout", [1, 2 * HID], F32)
    t_hb = nc.dram_tensor("hb", [128, cpr * HID], BF16)
    t_ag = nc.dram_tensor("agh", [M * 128, cpr * HID], BF16)
    groups = [list(range(M))]

    def loop(tc, body, active):
        if active and nloop > 1:
            with tc.For_i(0, nloop, 1):
                body()
        else:
            body()

    tc1 = TileContext(nc)
    with tc1 as tc:
        with (tc.tile_pool(name="const", bufs=1) as cpool,
              tc.tile_pool(name="work", bufs=5) as wpool,
              tc.tile_pool(name="hout", bufs=4) as hpool,
              tc.tile_pool(name="psZ", bufs=4, space="PSUM") as psZ,
              tc.tile_pool(name="psB", bufs=1, space="PSUM") as psB,
              tc.tile_pool(name="psS", bufs=1, space="PSUM") as psS,
              tc.tile_pool(name="small", bufs=1) as smpool):
            # ---- constants ----
            s1sb = []
            for r in range(2):
                t = cpool.tile([128, p.shard_pad], BF16, tag=f"s1{r}")
                nc.sync.dma_start(out=t[:], in_=t_s1T[r * 128:(r + 1) * 128, :])
                s1sb.append(t)
            w_f32, w_bf = [], []
            for r in range(2):
                w = cpool.tile([128, HID], F32, tag=f"wf{r}")
                nc.sync.dma_start(out=w[:], in_=t_wpre[r * 128:(r + 1) * 128, :])
                w_f32.append(w)
                wb = cpool.tile([128, HID], BF16, tag=f"wb{r}")
                nc.vector.tensor_copy(out=wb[:], in_=w[:])
                w_bf.append(wb)
            gamma_sb = cpool.tile([1, HID], F32, tag="gm")
            nc.sync.dma_start(out=gamma_sb[:], in_=t_gamma[:])
            beta_sb = cpool.tile([1, HID], F32, tag="bt")
            nc.sync.dma_start(out=beta_sb[:], in_=t_beta[:])
            ones1_sb = cpool.tile([1, 128], F32, tag="on")
            nc.sync.dma_start(out=ones1_sb[:], in_=t_ones1[:])
            onesc_sb = cpool.tile([128, 1], F32, tag="onc")
            nc.sync.dma_start(out=onesc_sb[:], in_=t_onesc[:])
            onesc_bf = cpool.tile([128, 1], BF16, tag="oncb")
            nc.vector.tensor_copy(out=onesc_bf[:], in_=onesc_sb[:])
            ones1_bf = cpool.tile([1, 128], BF16, tag="on16")
            nc.vector.tensor_copy(out=ones1_bf[:], in_=ones1_sb[:])

            # ---- P1: z -> SBUF store; then squares + column-sum matmuls ----
            sums = psS.tile([1, 2 * HID], F32, tag="sums", name="sums")
            zst = cpool.tile([128, cpr * HID], BF16, tag="zst")
            zqt = cpool.tile([128, cpr * HID], BF16, tag="zqt")
            def phase1():
                for j in range(cpr):
                    zc = psZ.tile([128, HID], F32, tag="zc")
                    co = j * 128
                    nc.tensor.matmul(zc[:], lhsT=s1sb[0][:, co:co + 128],
                                     rhs=w_bf[0][:], start=True, stop=False)
                    nc.tensor.matmul(zc[:], lhsT=s1sb[1][:, co:co + 128],
                                     rhs=w_bf[1][:], start=False, stop=True)
                    nc.vector.tensor_copy(out=zst[:, j * HID:(j + 1) * HID],
                                          in_=zc[:])
                for j in range(cpr):
                    nc.scalar.activation(out=zqt[:, j * HID:(j + 1) * HID],
                                         in_=zst[:, j * HID:(j + 1) * HID],
                                         func=mybir.ActivationFunctionType.Square,
                                         bias=0.0, scale=1.0)
                for j in range(cpr):
                    nc.tensor.matmul(sums[:, :HID], lhsT=onesc_bf[:],
                                     rhs=zst[:, j * HID:(j + 1) * HID],
                                     start=(j == 0), stop=(j == cpr - 1))
                    nc.tensor.matmul(sums[:, HID:], lhsT=onesc_bf[:],
                                     rhs=zqt[:, j * HID:(j + 1) * HID],
                                     start=(j == 0), stop=(j == cpr - 1))
            loop(tc, phase1, "1" in phases)

            # ---- AR: allreduce stats ----
            sums_sb = smpool.tile([1, 2 * HID], F32, tag="ssb")
            nc.vector.tensor_copy(out=sums_sb[:], in_=sums[:])
            nc.sync.dma_start(out=t_arin[:], in_=sums_sb[:])
            def phase_ar():
                nc.gpsimd.collective_compute(
                    "AllReduce", mybir.AluOpType.add, replica_groups=groups,
                    ins=[t_arin[:]], outs=[t_arout[:]])
            if coll:
                loop(tc, phase_ar, "r" in phases)
            stats_sb = smpool.tile([1, 2 * HID], F32, tag="stats")
            nc.sync.dma_start(out=stats_sb[:],
                              in_=(t_arout[:] if coll else t_arin[:]))

            # ---- stats finalize ----
            invn = 1.0 / p.N
            mu = smpool.tile([1, HID], F32, tag="mu")
            nc.vector.tensor_scalar(out=mu[:], in0=stats_sb[:, :HID], scalar1=invn,
                                    scalar2=None, op0=mybir.AluOpType.mult)
            var = smpool.tile([1, HID], F32, tag="var")
            nc.vector.tensor_scalar(out=var[:], in0=stats_sb[:, HID:], scalar1=invn,
                                    scalar2=None, op0=mybir.AluOpType.mult)
            musq = smpool.tile([1, HID], F32, tag="musq")
            nc.vector.tensor_tensor(out=musq[:], in0=mu[:], in1=mu[:],
                                    op=mybir.AluOpType.mult)
            nc.vector.tensor_tensor(out=var[:], in0=var[:], in1=musq[:],
                                    op=mybir.AluOpType.subtract)
            nc.vector.tensor_scalar(out=var[:], in0=var[:], scalar1=BN_EPS,
                                    scalar2=None, op0=mybir.AluOpType.add)
            sq = smpool.tile([1, HID], F32, tag="sq")
            nc.scalar.activation(out=sq[:], in_=var[:],
                                 func=mybir.ActivationFunctionType.Sqrt,
                                 bias=0.0, scale=1.0)
            rs = smpool.tile([1, HID], F32, tag="rs")
            nc.vector.reciprocal(out=rs[:], in_=sq[:])
            a_vec = smpool.tile([1, HID], F32, tag="av")
            nc.vector.tensor_tensor(out=a_vec[:], in0=rs[:], in1=gamma_sb[:],
                                    op=mybir.AluOpType.mult)
            b_vec = smpool.tile([1, HID], F32, tag="bv")
            nc.vector.tensor_tensor(out=b_vec[:], in0=mu[:], in1=a_vec[:],
                                    op=mybir.AluOpType.mult)
            nc.vector.tensor_tensor(out=b_vec[:], in0=beta_sb[:], in1=b_vec[:],
                                    op=mybir.AluOpType.subtract)
            ab_cat = smpool.tile([1, 2 * HID], F32, tag="abc")
            nc.vector.tensor_copy(out=ab_cat[:, :HID], in_=a_vec[:])
            nc.vector.tensor_copy(out=ab_cat[:, HID:], in_=b_vec[:])
            ps_ab = psB.tile([128, 2 * HID], F32, tag="zab", name="ps_ab")
            nc.tensor.matmul(ps_ab[:], lhsT=ones1_sb[:], rhs=ab_cat[:],
                             start=True, stop=True)
            a_bc = cpool.tile([128, HID], F32, tag="abc2")
            nc.vector.tensor_copy(out=a_bc[:], in_=ps_ab[:, :HID])
            b_bc = cpool.tile([128, HID], F32, tag="bbc2")
            nc.vector.tensor_copy(out=b_bc[:], in_=ps_ab[:, HID:])

            # ---- P3: h = relu(a*z + b) from stored z (DVE/Act only) ----
            HSPAN = 7
            def phase3():
                for j in range(cpr):
                    zj = zst[:, j * HID:(j + 1) * HID]
                    t1 = wpool.tile([128, HID], F32, tag="t1")
                    nc.vector.tensor_tensor(out=t1[:], in0=zj, in1=a_bc[:],
                                            op=mybir.AluOpType.mult)
                    hs = j // HSPAN
                    ho = j % HSPAN
                    he = min(cpr, (hs + 1) * HSPAN) - hs * HSPAN
                    if ho == 0:
                        hsp = hpool.tile([128, HSPAN * HID], BF16, tag="hsp",
                                         name=f"hsp_{hs % 4}")
                        phase3.hsp = hsp
                    hsp = phase3.hsp
                    hj = hsp[:, ho * HID:(ho + 1) * HID]
                    t2 = wpool.tile([128, HID], F32, tag="t2")
                    if j % 2 == 0:
                        nc.vector.tensor_tensor(out=t2[:], in0=t1[:], in1=b_bc[:],
                                                op=mybir.AluOpType.add)
                        nc.vector.tensor_scalar(out=hj, in0=t2[:], scalar1=0.0,
                                                scalar2=None,
                                                op0=mybir.AluOpType.max)
                    else:
                        nc.any.tensor_tensor(out=t2[:], in0=t1[:], in1=b_bc[:],
                                             op=mybir.AluOpType.add)
                        nc.scalar.activation(out=hj, in_=t2[:],
                                             func=mybir.ActivationFunctionType.Relu,
                                             bias=0.0, scale=1.0)
                    if ho == he - 1:
                        nc.sync.dma_start(
                            out=t_hb[:, hs * HSPAN * HID:(hs * HSPAN + he) * HID],
                            in_=hsp[:, :he * HID])
            loop(tc, phase3, "3" in phases)

            # ---- AG: allgather h shards ----
            def phase_ag():
                nc.gpsimd.collective_compute(
                    "AllGather", mybir.AluOpType.bypass, replica_groups=groups,
                    ins=[t_hb[:]], outs=[t_ag[:]])
            if coll:
                loop(tc, phase_ag, "G" in phases)

        # ---------------- phase C: aggregate + output ----------------
        with (tc.tile_pool(name="c2", bufs=1) as cpool,
              tc.tile_pool(name="gat", bufs=10) as gpool,
              tc.tile_pool(name="sel", bufs=16) as selp,
              tc.tile_pool(name="epi", bufs=3) as epool,
              tc.tile_pool(name="psG", bufs=4, space="PSUM") as psG,
              tc.tile_pool(name="psT", bufs=2, space="PSUM") as psT,
              tc.tile_pool(name="psO", bufs=2, space="PSUM") as psO):
            idx_sb = cpool.tile([128, ktot * 8], I16, tag="idx")
            nc.sync.dma_start(out=idx_sb[:], in_=t_idx[:])
            drel_sb = cpool.tile([128, ktot], F32, tag="dr")
            nc.sync.dma_start(out=drel_sb[:], in_=t_drel[:])
            iota_sb = cpool.tile([128, 128], BF16, tag="io")
            nc.sync.dma_start(out=iota_sb[:], in_=t_iota[:])
            ident_sb = cpool.tile([128, 128], F32, tag="idn")
            nc.sync.dma_start(out=ident_sb[:], in_=t_ident[:])
            ident_bf = cpool.tile([128, 128], BF16, tag="idnb")
            nc.vector.tensor_copy(out=ident_bf[:], in_=ident_sb[:])
            hsh = cpool.tile([128, cpr * HID], BF16, tag="hsh")
            nc.sync.dma_start(out=hsh[:], in_=t_hb[:])
            wop2 = []
            for r in range(2):
                w = cpool.tile([128, HID], F32, tag=f"wo2{r}")
                nc.sync.dma_start(out=w[:], in_=t_wop[r * 128:(r + 1) * 128, :])
                wb = cpool.tile([128, HID], BF16, tag=f"wo2b{r}")
                nc.vector.tensor_scalar(out=wb[:], in0=w[:], scalar1=W_OP_SCALE,
                                        scalar2=None, op0=mybir.AluOpType.mult)
                wop2.append(wb)

            half_parts = (M // 2) * 128
            tbl_lo = t_ag[0:half_parts, :]
            tbl_hi = t_ag[half_parts:2 * half_parts, :]

            qn = [0]
            ESPAN = 7
            nspan = math.ceil(p.tiles / ESPAN)
            sup_sp = [epool.tile([128, ESPAN * HID], F32, tag="sup",
                                 name=f"sup{es}") for es in range(nspan)]
            def phaseC():
                # loop 1: gather + segment-sum; agg -> sup spans (Act, x0.9)
                for t in range(p.tiles):
                    gt = {}
                    for hh in (0, 1) if 'g' in c_parts else ():
                        kk = int(K[t, hh])
                        g = gpool.tile([128, kk * HID], BF16, tag=f"g{hh}")
                        tbl = tbl_lo if hh == 0 else tbl_hi
                        o8 = int(off[t, hh]) * 8
                        nc.gpsimd.dma_gather(
                            out_ap=g[:].rearrange("p (c d) -> p c d", d=HID),
                            in_ap=tbl.rearrange("q (c d) -> (q c) d", d=HID),
                            idxs_ap=idx_sb[:, o8:o8 + kk * 8],
                            num_idxs=kk * 128, num_idxs_reg=kk * 128,
                            elem_size=HID, single_packet=False,
                            queue_num=qn[0] % 4)
                        qn[0] += 1
                        gt[hh] = g
                    agg = psG.tile([128, HID], F32, tag="agg")
                    nmm = int(K[t, 0] + K[t, 1]) + 1
                    ci = 0
                    if 's' not in c_parts or 'g' not in c_parts:
                        nc.vector.memset(agg[:], 0.0)
                    else:
                        # self term: agg += h_shard chunk t (exact bf16)
                        nc.tensor.matmul(agg[:], lhsT=ident_bf[:],
                                         rhs=hsh[:, t * HID:(t + 1) * HID],
                                         start=True, stop=False)
                        ci = 1
                    for hh in ((0, 1) if ('s' in c_parts and 'g' in c_parts) else ()):
                        kk = int(K[t, hh])
                        for c in range(kk):
                            col = int(off[t, hh]) + c
                            S = selp.tile([128, 128], BF16)
                            nc.vector.tensor_scalar(
                                out=S[:], in0=iota_sb[:],
                                scalar1=drel_sb[:, col:col + 1], scalar2=None,
                                op0=mybir.AluOpType.is_equal)
                            nc.tensor.matmul(agg[:], lhsT=S[:],
                                             rhs=gt[hh][:, c * HID:(c + 1) * HID],
                                             start=(ci == 0), stop=(ci == nmm - 1))
                            ci += 1
                    if 'e' not in c_parts:
                        continue
                    es, eo = t // ESPAN, t % ESPAN
                    nc.scalar.activation(out=sup_sp[es][:, eo * HID:(eo + 1) * HID],
                                         in_=agg[:],
                                         func=mybir.ActivationFunctionType.Copy,
                                         bias=0.0, scale=(1.0 - ALPHA))
                    e1 = min(p.tiles, (es + 1) * ESPAN)
                    if t == e1 - 1:
                        epilogue_span(es)
                if 'e' not in c_parts:
                    return
            def epilogue_span(es):
                e0 = es * ESPAN
                e1 = min(p.tiles, e0 + ESPAN)
                ne = e1 - e0
                x0sp = epool.tile([128, ESPAN * HID], F32, tag="x0sp",
                                  name=f"x0sp{es % 2}")
                nc.sync.dma_start(out=x0sp[:, :ne * HID],
                                  in_=t_x0[:, e0 * HID:e1 * HID])
                outsp = epool.tile([128, ESPAN * HID], F32, tag="outsp",
                                   name=f"outsp{es % 2}")
                sup = sup_sp[es]
                nc.vector.tensor_tensor(out=sup[:, :ne * HID],
                                        in0=sup[:, :ne * HID],
                                        in1=x0sp[:, :ne * HID],
                                        op=mybir.AluOpType.add)
                for t in range(e0, e1):
                    eo = t - e0
                    st = sup[:, eo * HID:(eo + 1) * HID]
                    trp = psT.tile([128, HID], F32, tag="tr")
                    for r in range(2):
                        nc.tensor.transpose(
                            out=trp[:, r * 128:(r + 1) * 128],
                            in_=st[:, r * 128:(r + 1) * 128],
                            identity=ident_sb[:])
                    supT = epool.tile([128, HID], BF16, tag="supT")
                    nc.any.tensor_copy(out=supT[:], in_=trp[:])
                    ops = psO.tile([128, HID], F32, tag="o")
                    nc.tensor.matmul(ops[:], lhsT=supT[:, :128], rhs=wop2[0][:],
                                     start=True, stop=False)
                    nc.tensor.matmul(ops[:], lhsT=supT[:, 128:], rhs=wop2[1][:],
                                     start=False, stop=True)
                    ut = epool.tile([128, HID], F32, tag="u")
                    nc.vector.tensor_tensor(out=ut[:], in0=st, in1=ops[:],
                                            op=mybir.AluOpType.add)
                    nc.scalar.activation(out=outsp[:, eo * HID:(eo + 1) * HID],
                                         in_=ut[:],
                                         func=mybir.ActivationFunctionType.Relu,
                                         bias=0.0, scale=OUT_SCALE)
                nc.sync.dma_start(out=t_out[:, e0 * HID:e1 * HID],
                                  in_=outsp[:, :ne * HID])
            loop(tc, phaseC, "C" in phases)

    nc.compile()
    return nc


def make_weight_inputs(prob, W_pre, gamma, beta_bn, W_op):
    return dict(
        wpre=np.asarray(W_pre, np.float32),
        gamma=np.asarray(gamma, np.float32).reshape(1, -1),
        beta=np.asarray(beta_bn, np.float32).reshape(1, -1),
        wop=np.asarray(W_op, np.float32),
    )


def unpack_out(prob, arr):
    return arr.reshape(128, prob.tiles, prob.HID).transpose(1, 0, 2).reshape(
        prob.shard_pad, prob.HID)


# ======================================================================
# Self-contained execution via PJRT (axon)
# ======================================================================
import jax
from jax.sharding import Mesh, PartitionSpec, NamedSharding
from jax.experimental.shard_map import shard_map
from concourse.bass2jax import _bass_exec_p, install_neuronx_cc_hook, partition_id_tensor


def _build_exec(nc, n_cores):
    install_neuronx_cc_hook()
    partition_name = nc.partition_id_tensor.name if nc.partition_id_tensor else None
    in_names, out_names, out_avals, zero_outs = [], [], [], []
    for alloc in nc.m.functions[0].allocations:
        if not isinstance(alloc, mybir.MemoryLocationSet):
            continue
        name = alloc.memorylocations[0].name
        if alloc.kind == "ExternalInput":
            if name != partition_name:
                in_names.append(name)
        elif alloc.kind == "ExternalOutput":
            shape = tuple(alloc.tensor_shape)
            dtype = mybir.dt.np(alloc.dtype)
            out_names.append(name)
            out_avals.append(jax.core.ShapedArray(shape, dtype))
            zero_outs.append(np.zeros(shape, dtype))
    n_params = len(in_names)
    n_outs = len(out_avals)
    all_in_names = list(in_names) + list(out_names)
    if partition_name is not None:
        all_in_names.append(partition_name)

    def _body(*args):
        operands = list(args)
        if partition_name is not None:
            operands.append(partition_id_tensor())
        outs = _bass_exec_p.bind(
            *operands, out_avals=tuple(out_avals), in_names=tuple(all_in_names),
            out_names=tuple(out_names), lowering_input_output_aliases=(),
            sim_require_finite=True, sim_require_nnan=True, nc=nc)
        return tuple(outs)

    devices = jax.devices()[:n_cores]
    mesh = Mesh(np.asarray(devices), ("core",))
    in_specs = (PartitionSpec("core"),) * (n_params + n_outs)
    out_specs = (PartitionSpec("core"),) * n_outs
    donate = tuple(range(n_params, n_params + n_outs))
    fn = jax.jit(shard_map(_body, mesh=mesh, in_specs=in_specs,
                           out_specs=out_specs, check_rep=False),
                 donate_argnums=donate, keep_unused=True)
    return dict(fn=fn, in_names=in_names, out_names=out_names,
                out_avals=out_avals, zero_outs=zero_outs, mesh=mesh,
                n_cores=n_cores)


def _place_inputs(ex, in_maps):
    sh = NamedSharding(ex["mesh"], PartitionSpec("core"))
    n_cores = ex["n_cores"]
    return [jax.device_put(
        np.concatenate([np.asarray(in_maps[c][name]) for c in range(n_cores)], axis=0), sh)
        for name in ex["in_names"]]


def _run(ex, dev_in):
    sh = NamedSharding(ex["mesh"], PartitionSpec("core"))
    n_cores = ex["n_cores"]
    zs = [jax.device_put(np.zeros((n_cores * z.shape[0], *z.shape[1:]), z.dtype), sh)
          for z in ex["zero_outs"]]
    outs = jax.block_until_ready(ex["fn"](*dev_in, *zs))
    return [
        {name: np.asarray(outs[i]).reshape(n_cores, *ex["out_avals"][i].shape)[c]
         for i, name in enumerate(ex["out_names"])}
        for c in range(n_cores)
    ]


_CACHE = {}


def _get_compiled(prob, meta, W_pre, gamma, beta_bn, W_op, key):
    if key not in _CACHE:
        nc = build_kernel(prob, meta, W_pre, gamma, beta_bn, W_op, nloop=1)
        _CACHE[key] = _build_exec(nc, prob.n_cores)
    return _CACHE[key]


def kernel(s0=None, s1=None, x_0=None, W_pre=None, gamma=None, beta_bn=None,
           W_op=None, edge_index=None, drop_prob=None, training=None, **_ignored):
    s1 = np.asarray(s1, np.float32)
    x_0 = np.asarray(x_0, np.float32)
    W_pre = np.asarray(W_pre, np.float32)
    gamma = np.asarray(gamma, np.float32)
    beta_bn = np.asarray(beta_bn, np.float32)
    W_op = np.asarray(W_op, np.float32)
    edge_index = np.asarray(edge_index)
    N, C = s1.shape
    HID = W_pre.shape[1]
    E = edge_index.shape[1]
    prob = Prob(N, E, C, HID, n_cores=8)
    in_maps, meta = host_prep(prob, s1, x_0, edge_index)
    key = (N, E, C, HID, int(np.int64(edge_index[:, ::97]).sum()), meta["ktot"])
    ex = _get_compiled(prob, meta, W_pre, gamma, beta_bn, W_op, key)
    wins = make_weight_inputs(prob, W_pre, gamma, beta_bn, W_op)
    full_maps = [{**m, **wins} for m in in_maps]
    dev_in = _place_inputs(ex, full_maps)
    res = _run(ex, dev_in)
    out = np.concatenate(
        [unpack_out(prob, res[m]["out"])[:prob.shard] for m in range(prob.n_cores)],
        axis=0)
    return np.ascontiguousarray(out[:N]).astype(np.float32)
